# revision 47
# baseline (speedup 1.0000x reference)
"""Multi-head attention (B=8, H=8, S=1024, d=128) on 8 TRN2 NeuronCores.

Strategy (v3)
-------------
- Job sharding: the 64 (batch, head) attention jobs are dealt so core c
  handles head c of ALL 8 batches. Every core then sees the same
  per-batch key-tile counts (compile shape = tuple(kt_b)), so keys are
  padded to each batch's own count (sum(kt_b) k-tiles per core) instead
  of the global max (8*max kt) -- ~10% less exp/matmul/DMA work.
- Host-side prep (layout only): per batch, compact keys/values to the
  seq_mask-selected rows (zero-padded per batch to kt_b 128-wide
  k-tiles), pre-transpose Q and K so the contraction dim lands on SBUF
  partitions, cast matmul operands to fp16. An indicator matrix
  ind[k, 32] rides along for the softmax denominator.
- All inputs are bulk-preloaded into SBUF (job-0 slices first on the
  HWDGE sync queue, the rest as 4 big DMAs on the gpsimd SWDGE queue);
  no per-job DMA stalls. A short burst of dummy matmuls warms the PE
  HAM clock-gate while the first DMAs land.
- Device math is a single software-pipelined stream over all k-tiles
  of all jobs. Cycle i runs concurrently on three engines:
    PE : QK  logitsT[k,q] = K^T[:, tile i+1].T @ Q^T   (fp16, 2 paired
         M=64 matmuls per q-half, diagonal PSUM banks)
    ACT: W^T = exp(logitsT * d^-0.5)  for tile i        (the pacer)
    PE : outT[d,q] += V.T @ W^T ; den[q] += ind.T @ W^T for tile i-1
  The one-cycle lag on the AV/den matmuls means every instruction's
  semaphores are long settled when it issues -- no engine round-trip
  stalls; ACT runs back-to-back at (1024+352)/1.2GHz per tile.
- Outputs leave as fp16 (numerator, and denominators packed 2 jobs per
  PSUM bank); the division happens on the host. The learned scalar
  bias b cancels in softmax. Fully-masked batches fall back to the
  uniform average on the host.
"""
from contextlib import ExitStack

import numpy as np

import concourse.bacc as bacc
import concourse.mybir as mybir
import concourse.tile as tile
from concourse.bass_utils import run_bass_kernel_spmd

F32 = mybir.dt.float32
F16 = mybir.dt.float16

B, S, D, H = 8, 1024, 1024, 8
DH = D // H              # 128, head dim = one partition tile
SCALE = float(DH) ** -0.5
NJ = 8                   # jobs per core (one per batch)

_NC_CACHE: dict[tuple, object] = {}

# build options (overridable for profiling experiments)
OPTS: dict = {}


def _build(kts: tuple, opts: dict | None = None):
    """Build + compile the per-core kernel; kts[j] = k-tiles of job j."""
    opts = opts or {}
    n_warm = opts.get("n_warm", 16)
    KT_TOT = sum(kts)
    K_TOT = KT_TOT * 128
    koff = [sum(kts[:j]) for j in range(NJ)]     # k-tile offset per job

    nc = bacc.Bacc("TRN2", target_bir_lowering=False, debug=False)

    # all inputs host-prearranged partition-major: dense [128, N] DMAs
    q_t = nc.dram_tensor("q_t", [128, NJ * S], F16, kind="ExternalInput")
    k_t = nc.dram_tensor("k_t", [128, K_TOT], F16, kind="ExternalInput")
    v_c = nc.dram_tensor("v_c", [128, KT_TOT * 128], F16,
                         kind="ExternalInput")
    ind = nc.dram_tensor("ind", [128, KT_TOT * 32], F16,
                         kind="ExternalInput")
    out_t = nc.dram_tensor("out_t", [128, NJ * S], F16, kind="ExternalOutput")
    den_t = nc.dram_tensor("den_t", [NJ // 2, 128, 512], F16,
                           kind="ExternalOutput")

    # input DMA regions: job 0 alone (compute starts as soon as it lands),
    # then two bulk groups. Each region gets its OWN SBUF tile per tensor:
    # Tile's DMA dependency tracking is tile-granular, so a shared tile
    # would make job 0 wait on the bulk transfers.
    regions = [(0, 1), (1, 4), (4, 8)]
    NR = len(regions)
    rof = [koff[j0] for j0, _ in regions]            # k-tile offset of region
    rkt = [sum(kts[j0:j1]) for j0, j1 in regions]    # k-tiles in region

    def region_of(j):
        for ri, (j0, j1) in enumerate(regions):
            if j0 <= j < j1:
                return ri, j0
        raise AssertionError(j)

    with tile.TileContext(nc) as tc, ExitStack() as ctx:
        pools = {}
        for nm in ("k", "q", "v", "i"):
            for ri in range(NR):
                pools[nm, ri] = ctx.enter_context(
                    tc.tile_pool(name=f"sb_{nm}{ri}", bufs=1))
        sb_ka = ctx.enter_context(tc.tile_pool(name="sb_ka", bufs=1))
        sb_wu = ctx.enter_context(tc.tile_pool(name="sb_wu", bufs=1))
        # one wt buffer per stream cycle: no pool-reuse guards on the
        # pacing ACTIVATE instructions (SBUF is plentiful)
        sb_w = ctx.enter_context(tc.tile_pool(name="sb_w", bufs=KT_TOT))
        sb_out = ctx.enter_context(tc.tile_pool(name="sb_out", bufs=NJ))
        sb_den = ctx.enter_context(tc.tile_pool(name="sb_den", bufs=NJ // 2))
        # PSUM budget (8 banks): pl 2x2 + po_s0 2x1 + po_s1 1 + pd 1 = 8.
        # po_s0 is double-buffered so a new job's first AV matmuls never
        # wait on the previous job's PSUM->SBUF copy.
        ps_l = ctx.enter_context(tc.tile_pool(name="ps_l", bufs=2, space="PSUM"))
        ps_o0 = ctx.enter_context(
            tc.tile_pool(name="ps_o0", bufs=2, space="PSUM"))
        ps_o1 = ctx.enter_context(
            tc.tile_pool(name="ps_o1", bufs=1, space="PSUM"))
        ps_d = ctx.enter_context(tc.tile_pool(name="ps_d", bufs=1, space="PSUM"))

        # ---- PE warm-up: dummy matmuls (on whatever SBUF holds) keep the
        # PE busy while the first input DMAs land, so HAM un-throttles
        # early. Nothing reads the results; the scratch PSUM bank is the
        # den bank, which job 0's first den matmul clears (start=True).
        wu = sb_wu.tile([128, 512], F16)
        nc.gpsimd.memset(wu[:], 0)
        plw = ps_d.tile([128, 512], F32, tag="pd", name="plw")
        for _ in range(n_warm):
            # short N=256 matmuls: fine-grained, so the first real QK
            # slots in with minimal queue delay once its data lands
            nc.tensor.matmul(plw[0:64, 0:256], wu[:, 0:64], wu[:, 0:256],
                             start=True, stop=True, skip_group_check=True)

        # ---- bulk input preload, all on the sync HWDGE queue (FIFO, so
        # completion follows issue order with no cross-queue round-robin).
        # Order by need-time: K/Q of regions 0-1 (QK is the stream head),
        # then V/ind of regions 0-1 (only needed LAG cycles later), then
        # region 2. All transfers are dense [128, N] copies.
        ktile, qtile, vtile, itile = {}, {}, {}, {}
        for ri, (j0, j1) in enumerate(regions):
            kc = rkt[ri] * 128
            ktile[ri] = pools["k", ri].tile([128, kc], F16, name=f"ktl{ri}")
            qtile[ri] = pools["q", ri].tile([128, (j1 - j0) * S], F16,
                                            name=f"qtl{ri}")
            vtile[ri] = pools["v", ri].tile([128, kc], F16, name=f"vtl{ri}")
            itile[ri] = pools["i", ri].tile([128, rkt[ri] * 32], F16,
                                            name=f"itl{ri}")
        ktile["A"] = sb_ka.tile([128, 128], F16, name="ktlA")

        def dma_kq(ri):
            j0, j1 = regions[ri]
            ks, qs = rof[ri] * 128, j0 * S
            nc.sync.dma_start(qtile[ri][:],
                              q_t.ap()[:, qs:qs + (j1 - j0) * S])
            nc.sync.dma_start(ktile[ri][:],
                              k_t.ap()[:, ks:ks + rkt[ri] * 128])

        def dma_vi(ri):
            ks, is_ = rof[ri] * 128, rof[ri] * 32
            nc.sync.dma_start(vtile[ri][:],
                              v_c.ap()[:, ks:ks + rkt[ri] * 128])
            nc.sync.dma_start(itile[ri][:],
                              ind.ap()[:, is_:is_ + rkt[ri] * 32])

        nc.sync.dma_start(ktile["A"][:], k_t.ap()[:, 0:128])
        dma_kq(0)
        dma_kq(1)
        dma_vi(0)
        dma_vi(1)
        dma_kq(2)
        dma_vi(2)

        s0, s1 = slice(0, 512), slice(512, 1024)

        # flat stream of (job, ktile) cycles
        cyc = [(j, kt) for j in range(NJ) for kt in range(kts[j])]
        N = len(cyc)
        pls, wts, po, pd = {}, {}, {}, {}
        stash = []
        assert all(k >= 3 for k in kts)

        def emit_qk(i):
            j, kt = cyc[i]
            ri, j0 = region_of(j)
            kth = ktile[ri]
            pl = ps_l.tile([128, S], F32, tag="pl", name=f"pl_{i}")
            ks = (koff[j] - rof[ri] + kt) * 128
            if i == 0:
                kth, ks = ktile["A"], 0
            kA, kB = slice(ks, ks + 64), slice(ks + 64, ks + 128)
            qs = (j - j0) * S
            qth = qtile[ri][:, qs:qs + S]
            nc.tensor.matmul(pl[0:64, s0], kth[:, kA], qth[:, s0])
            nc.tensor.matmul(pl[64:128, s1], kth[:, kB], qth[:, s1])
            nc.tensor.matmul(pl[64:128, s0], kth[:, kB], qth[:, s0])
            nc.tensor.matmul(pl[0:64, s1], kth[:, kA], qth[:, s1])
            pls[i] = pl

        def emit_exp(i):
            wt = sb_w.tile([128, S], F16, tag="wt", name=f"wt_{i}")
            if opts.get("imm_bias", True):
                # bass forces a const-AP bias for Exp; emit the raw
                # instruction with immediate bias/scale/alpha instead so
                # the engine skips the per-instruction bias AP read
                eng = nc.scalar
                imm = lambda v: mybir.ImmediateValue(
                    dtype=mybir.dt.float32, value=v)
                eng.add_instruction(mybir.InstActivation(
                    name=nc.get_next_instruction_name(),
                    func=mybir.ActivationFunctionType.Exp,
                    ins=[eng.lower_ap(pls.pop(i)[:]), imm(0.0), imm(SCALE),
                         imm(0.0)],
                    outs=[eng.lower_ap(wt[:])]))
            else:
                nc.scalar.activation(
                    wt[:], pls.pop(i)[:], mybir.ActivationFunctionType.Exp,
                    scale=SCALE)
            wts[i] = wt

        def emit_avden(i):
            j, kt = cyc[i]
            first, last = kt == 0, kt == kts[j] - 1
            if first:
                po[j] = (ps_o0.tile([128, 512], F32, tag="po0",
                                    name=f"po0_{j}"),
                         ps_o1.tile([128, 512], F32, tag="po1",
                                    name=f"po1_{j}"))
                if j % 2 == 0:
                    pd[j // 2] = ps_d.tile([128, 512], F32, tag="pd",
                                           name=f"pd_{j // 2}")
            r0 = (j % 2) * 64
            pdj = pd[j // 2]
            p0, p1 = po[j]
            wt = wts.pop(i)
            ri, _ = region_of(j)
            t = koff[j] - rof[ri] + kt
            ic = slice(t * 32, t * 32 + 32)
            ind_sb = itile[ri]
            vA = vtile[ri][:, t * 128:t * 128 + 64]
            vB = vtile[ri][:, t * 128 + 64:t * 128 + 128]
            # flush stashed matmuls whose delay has elapsed (work deferred
            # from a job's first k-tile so PSUM handoffs get extra slack
            # without overloading any single PE cycle)
            for ent in list(stash):
                ent[0] -= 1
                if ent[0] <= 0:
                    for out_ap, w_ap, r_ap, tp, st in ent[1]:
                        nc.tensor.matmul(out_ap, w_ap, r_ap, start=st,
                                         stop=False, tile_position=tp,
                                         skip_group_check=True)
                    stash.remove(ent)
            if first:
                # emit only the s0 pair now (po_s0 is double-buffered so
                # it is free); defer den one cycle (pd copy in flight on
                # even jobs) and the s1 pair two cycles (po_s1 copy).
                # The s1 PSUM clear (start=True) moves to kt1's s1 pair,
                # which executes first; accumulation order is free.
                seqs = [
                    (p0[0:64, :], vA, wt[:, s0], None, True),
                    (p0[64:128, :], vB, wt[:, s0], None, True),
                ]
                stash.append([1, [
                    (pdj[r0:r0 + 32, :], ind_sb[:, ic], wt[:, s0],
                     (0, r0), True),
                    (pdj[r0 + 32:r0 + 64, :], ind_sb[:, ic], wt[:, s1],
                     (0, r0 + 32), True),
                ]])
                stash.append([2, [
                    (p1[64:128, :], vB, wt[:, s1], None, False),
                    (p1[0:64, :], vA, wt[:, s1], None, False),
                ]])
            else:
                s1st = kt == 1   # kt1 carries po_s1's accumulation start
                seqs = [
                    (pdj[r0:r0 + 32, :], ind_sb[:, ic], wt[:, s0],
                     (0, r0), False),
                    (pdj[r0 + 32:r0 + 64, :], ind_sb[:, ic], wt[:, s1],
                     (0, r0 + 32), False),
                    (p0[0:64, :], vA, wt[:, s0], None, False),
                    (p1[64:128, :], vB, wt[:, s1], None, s1st),
                    (p0[64:128, :], vB, wt[:, s0], None, False),
                    (p1[0:64, :], vA, wt[:, s1], None, s1st),
                ]
            for out_ap, w_ap, r_ap, tp, st in seqs:
                nc.tensor.matmul(out_ap, w_ap, r_ap, start=st, stop=last,
                                 tile_position=tp, skip_group_check=True)
            if last:
                # split copies; po halves release independently
                osb = sb_out.tile([128, S], F16, tag="osb", name=f"osb_{j}")
                p0, p1 = po.pop(j)
                if j == NJ - 1:
                    # ACT is idle after the last exp: split the final
                    # copies across both engines and DMA per half
                    nc.scalar.copy(osb[:, s1], p1[:])
                    nc.vector.tensor_copy(osb[:, s0], p0[:])
                    nc.sync.dma_start(out_t.ap()[:, j * S + 512:(j + 1) * S],
                                      osb[:, s1])
                    nc.sync.dma_start(out_t.ap()[:, j * S:j * S + 512],
                                      osb[:, s0])
                else:
                    nc.vector.tensor_copy(osb[:, s1], p1[:])
                    nc.vector.tensor_copy(osb[:, s0], p0[:])
                    nc.sync.dma_start(out_t.ap()[:, j * S:(j + 1) * S], osb[:])
                if j % 2 == 1:
                    dsb = sb_den.tile([128, 512], F16, tag="dsb",
                                      name=f"dsb_{j // 2}")
                    if j == NJ - 1:
                        nc.scalar.copy(dsb[:], pd.pop(j // 2)[:])
                    else:
                        nc.vector.tensor_copy(dsb[:], pd.pop(j // 2)[:])
                    nc.sync.dma_start(den_t.ap()[j // 2, :, :], dsb[:])

        # software-pipelined stream: cycle i = QK(i+1) | exp(i) | AVden(i-2)
        # (the AV lag keeps every matmul's semaphores settled at issue)
        LAG = 2
        emit_qk(0)
        for i in range(N):
            if i + 1 < N:
                emit_qk(i + 1)
            emit_exp(i)
            if i >= LAG:
                emit_avden(i - LAG)
        for i in range(N - LAG, N):
            emit_avden(i)

    nc.compile()
    return nc


def kernel(memory, query, seq_mask, b):
    memory = np.ascontiguousarray(memory, dtype=np.float32)
    query = np.ascontiguousarray(query, dtype=np.float32)
    seq_mask = np.asarray(seq_mask)
    assert memory.shape == (B, S, 2 * D) and query.shape == (B, S, D)

    counts = [int(np.count_nonzero(seq_mask[i])) for i in range(B)]
    kps = [max(((c + 127) // 128) * 128, 128) for c in counts]
    kts = tuple(kp // 128 for kp in kps)
    K_TOT = sum(kps)

    key = (kts, tuple(sorted(OPTS.items())))
    if key not in _NC_CACHE:
        _NC_CACHE[key] = _build(kts, OPTS)
    nc = _NC_CACHE[key]

    # shared compacted K/V/ind, stacked per batch along k
    KT_TOT = K_TOT // 128
    k_allT = np.zeros((D, K_TOT), dtype=np.float16)
    v_all = np.zeros((K_TOT, D), dtype=np.float16)
    ind_all = np.zeros((K_TOT, 32), dtype=np.float16)
    off = 0
    for i in range(B):
        idx = np.flatnonzero(seq_mask[i])
        nb = len(idx)
        if nb:
            k_allT[:, off:off + nb] = memory[i, idx, :D].T
            v_all[off:off + nb] = memory[i, idx, D:]
            ind_all[off:off + nb] = 1.0
        off += kps[i]

    # partition-major layouts so every DMA is a dense [128, N] row copy
    ind_pm = np.ascontiguousarray(
        ind_all.reshape(KT_TOT, 128, 32).transpose(1, 0, 2).reshape(128, -1))
    v_pm = v_all.reshape(KT_TOT, 128, D).transpose(1, 0, 2)  # [128, t, D]

    q_t = query.transpose(0, 2, 1).astype(np.float16)     # [B, D, S]
    in_maps = []
    for c in range(B):
        hs = c * DH
        qc = np.ascontiguousarray(
            q_t[:, hs:hs + DH, :].transpose(1, 0, 2).reshape(DH, B * S))
        kc = np.ascontiguousarray(k_allT[hs:hs + DH])
        vc = np.ascontiguousarray(
            v_pm[:, :, hs:hs + DH].reshape(128, KT_TOT * DH))
        in_maps.append({"q_t": qc, "k_t": kc, "v_c": vc, "ind": ind_pm})

    res = run_bass_kernel_spmd(nc, in_maps, list(range(B)))
    out = np.empty((B, S, D), dtype=np.float32)
    for c in range(B):
        hs = c * DH
        num = res.results[c]["out_t"].astype(np.float32)   # [DH, B*S]
        dd = res.results[c]["den_t"].astype(np.float32)    # [4, 128, 512]
        for j in range(B):
            blk = dd[j // 2]
            r0 = (j % 2) * 64
            den = np.concatenate([blk[r0], blk[r0 + 32]])  # [S]
            with np.errstate(divide="ignore", invalid="ignore"):
                out[j, :, hs:hs + DH] = (num[:, j * S:(j + 1) * S] /
                                         den[None, :]).T
    for i in range(B):
        if counts[i] == 0:
            out[i] = memory[i, :, D:].mean(axis=0)[None, :]
    return out
